# revision 21
# baseline (speedup 1.0000x reference)
"""1-D nearest-neighbor retrieval kernel for Trainium2 (8 NeuronCores).

For each query x[b], finds argmin_n |input_tensor[n] - x[b]| and returns
accuracy_tensor[argmin].  Queries are sharded across the 8 cores (512 each);
the reference/accuracy tables are replicated.

Per-core pipeline (queries in SBUF partitions, refs in the free dim):
  Phase 1 -- segment minima:
    - PE computes diff[p, j] = ref_j - x_p into PSUM with two accumulating
      K=1 matmuls: ones.T @ ref_row then x_row.T @ (-ones_row).  Each
      product has a +-1.0 factor (exact); the single PSUM accumulation is
      one fp32 add, so diff is bit-identical to fp32(ref - x).
    - VectorE reduces each 128-wide segment to min |diff| in one fused
      instruction (tensor_reduce min with apply_absolute_value).
  Phase 2 -- exact argmin from segment minima (per query tile):
    - global min m = reduce_min(seg); first segment with seg == m via
      max_index; indirect-DMA gather of that segment's refs per lane;
      recompute |ref - x| (bit-identical); max_index for the in-segment
      position.  First-occurrence semantics match argmin's tie-break.
    - indirect-DMA gather of accuracy_tensor[argmin].

Engine instructions are kept to <=1 cross-engine semaphore wait (the
walrus codegen rejects more on most instruction structs).
"""
from contextlib import ExitStack

import numpy as np

import concourse.bass as bass
import concourse.bacc as bacc
import concourse.tile as tile
from concourse import mybir
from concourse._compat import with_exitstack
from concourse.bass_utils import run_bass_kernel_spmd

P = 128
N_CORES = 8
B = 4096
B_CORE = B // N_CORES  # 512
N = 65536
F = 2048               # refs per chunk (4 PSUM banks)
N_CHUNKS = N // F      # 32
N_QT = B_CORE // P     # 4 query tiles per core
W = 128                # segment width
S = F // W             # 16 segments per chunk
NSEG = N // W          # 512 segments total
MM = 512               # max moving free dim per matmul

FP32 = mybir.dt.float32
U32 = mybir.dt.uint32
I32 = mybir.dt.int32


@with_exitstack
def _nn_kernel(ctx: ExitStack, tc: tile.TileContext, xq, refs, acc, out):
    nc = tc.nc

    row_pool = ctx.enter_context(tc.tile_pool(name="row", bufs=3))
    psum_pool = ctx.enter_context(tc.tile_pool(name="psum", bufs=2, space="PSUM"))
    small_pool = ctx.enter_context(tc.tile_pool(name="small", bufs=2))
    persist = ctx.enter_context(tc.tile_pool(name="persist", bufs=1))

    # Queries, [128, 4] (partition-major) for phase 2 and [4, 128]
    # (row-major) as matmul stationaries.
    x_sb = persist.tile([P, N_QT], FP32, tag="x_sb")
    nc.sync.dma_start(out=x_sb[:], in_=xq.rearrange("(q p) -> p q", p=P))
    x_rows = []
    for qt in range(N_QT):
        xr = persist.tile([1, P], FP32, tag=f"x_row{qt}", name=f"x_row{qt}")
        nc.sync.dma_start(out=xr[:], in_=xq[None, qt * P : (qt + 1) * P])
        x_rows.append(xr)

    ones = nc.const_aps.tensor(1.0, [1, P])
    neg1_row = persist.tile([1, F], FP32, tag="neg1_row")
    nc.vector.memset(neg1_row[:], -1.0)

    # Per-qtile segment minima, filled chunk by chunk.
    segs = [
        persist.tile([P, NSEG], FP32, tag=f"seg{qt}", name=f"seg{qt}")
        for qt in range(N_QT)
    ]

    # PE warmups: observe each setup dependency (neg1_row memset, x_row
    # DMAs) with a single-wait dummy matmul, so the phase-1 matmuls carry
    # at most one fresh semaphore wait each.
    warm = psum_pool.tile([P, F], FP32, tag="diff")
    nc.tensor.matmul(
        out=warm[:, :P], lhsT=ones, rhs=neg1_row[:, :P], start=True, stop=True
    )
    for qt in range(N_QT):
        nc.tensor.matmul(
            out=warm[:, :P], lhsT=ones, rhs=x_rows[qt][:], start=True, stop=True
        )

    # ---- Phase 1: segment minima ----
    for c in range(N_CHUNKS):
        row = row_pool.tile([1, F], FP32, tag="row")
        nc.sync.dma_start(out=row[:], in_=refs[None, c * F : (c + 1) * F])
        for qt in range(N_QT):
            diff = psum_pool.tile([P, F], FP32, tag="diff")
            # diff = (-x_p) then += ref_j: each product has a +-1.0 factor
            # (exact); the PSUM accumulation is one fp32 add, so diff is
            # bit-identical to fp32(ref - x).  Stationary-major order keeps
            # weight loads to two per tile.
            for j in range(F // MM):
                nc.tensor.matmul(
                    out=diff[:, j * MM : (j + 1) * MM],
                    lhsT=x_rows[qt][:],
                    rhs=neg1_row[:, j * MM : (j + 1) * MM],
                    start=True,
                    stop=False,
                )
            for j in range(F // MM):
                nc.tensor.matmul(
                    out=diff[:, j * MM : (j + 1) * MM],
                    lhsT=ones,
                    rhs=row[:, j * MM : (j + 1) * MM],
                    start=False,
                    stop=True,
                )
            nc.vector.tensor_reduce(
                segs[qt][:, c * S : (c + 1) * S],
                diff[:].rearrange("p (s w) -> p s w", w=W),
                axis=mybir.AxisListType.X,
                op=mybir.AluOpType.min,
                apply_absolute_value=True,
            )

    # ---- Phase 2: exact argmin per query tile ----
    refs2d = refs.rearrange("(s w) -> s w", w=W)
    for qt in range(N_QT):
        gmin = small_pool.tile([P, 1], FP32, tag="gmin")
        nc.vector.tensor_reduce(
            gmin[:], segs[qt][:], axis=mybir.AxisListType.X, op=mybir.AluOpType.min
        )
        m8 = small_pool.tile([P, 8], FP32, tag="m8")
        nc.vector.tensor_copy(m8[:], gmin[:, 0:1].to_broadcast([P, 8]))
        s8 = small_pool.tile([P, 8], U32, tag="s8")
        nc.vector.max_index(s8[:], m8[:], segs[qt][:])
        seg_f = small_pool.tile([P, 1], FP32, tag="seg_f")
        nc.vector.tensor_copy(seg_f[:], s8[:, 0:1])
        seg_i = small_pool.tile([P, 1], I32, tag="seg_i")
        nc.vector.tensor_copy(seg_i[:], seg_f[:])
        # Gather the winning 128-ref segment for each lane.
        gref = small_pool.tile([P, W], FP32, tag="gref")
        nc.gpsimd.indirect_dma_start(
            out=gref[:],
            out_offset=None,
            in_=refs2d,
            in_offset=bass.IndirectOffsetOnAxis(ap=seg_i[:, 0:1], axis=0),
        )
        # Recompute ref - x for the gathered segment (bit-identical signed
        # diff) and search it for +gmin / -gmin; the smaller found index is
        # the first position with |diff| == gmin.
        dist_w = small_pool.tile([P, W], FP32, tag="dist_w")
        nc.vector.tensor_scalar(
            dist_w[:],
            gref[:],
            x_sb[:, qt : qt + 1],
            None,
            op0=mybir.AluOpType.subtract,
        )
        mpm = small_pool.tile([P, 8], FP32, tag="mpm")
        nc.vector.tensor_copy(mpm[:, 0:4], gmin[:, 0:1].to_broadcast([P, 4]))
        nc.vector.tensor_scalar(
            mpm[:, 4:8],
            gmin[:, 0:1].to_broadcast([P, 4]),
            -1.0,
            None,
            op0=mybir.AluOpType.mult,
        )
        w8 = small_pool.tile([P, 8], U32, tag="w8")
        nc.vector.max_index(w8[:], mpm[:], dist_w[:])
        # Global index = seg * W + within-segment index (fp32 arithmetic is
        # exact for values < 2^24; a not-found slot becomes 2^32-1 in fp32
        # and loses the min).
        wp_f = small_pool.tile([P, 1], FP32, tag="wp_f")
        nc.vector.tensor_copy(wp_f[:], w8[:, 0:1])
        wm_f = small_pool.tile([P, 1], FP32, tag="wm_f")
        nc.vector.tensor_copy(wm_f[:], w8[:, 4:5])
        w_f = small_pool.tile([P, 1], FP32, tag="w_f")
        nc.vector.tensor_tensor(
            out=w_f[:], in0=wp_f[:], in1=wm_f[:], op=mybir.AluOpType.min
        )
        idx_f = small_pool.tile([P, 1], FP32, tag="idx_f")
        nc.vector.tensor_scalar(
            idx_f[:], seg_f[:], float(W), None, op0=mybir.AluOpType.mult
        )
        nc.vector.tensor_tensor(
            out=idx_f[:], in0=idx_f[:], in1=w_f[:], op=mybir.AluOpType.add
        )
        idxg = small_pool.tile([P, 1], I32, tag="idxg")
        nc.vector.tensor_copy(idxg[:], idx_f[:])
        acc_g = small_pool.tile([P, 1], FP32, tag="acc_g")
        nc.gpsimd.indirect_dma_start(
            out=acc_g[:],
            out_offset=None,
            in_=acc[:, None],
            in_offset=bass.IndirectOffsetOnAxis(ap=idxg[:, 0:1], axis=0),
        )
        nc.sync.dma_start(out=out[qt * P : (qt + 1) * P, None], in_=acc_g[:])


_CACHED_NC = None


def _build():
    global _CACHED_NC
    if _CACHED_NC is not None:
        return _CACHED_NC
    nc = bacc.Bacc("TRN2", target_bir_lowering=False, debug=False)
    xq = nc.dram_tensor("xq", [B_CORE], FP32, kind="ExternalInput").ap()
    refs = nc.dram_tensor("refs", [N], FP32, kind="ExternalInput").ap()
    acc = nc.dram_tensor("acc", [N], FP32, kind="ExternalInput").ap()
    out = nc.dram_tensor("out", [B_CORE], FP32, kind="ExternalOutput").ap()
    with tile.TileContext(nc) as tc:
        _nn_kernel(tc, xq, refs, acc, out)
    nc.compile()
    _CACHED_NC = nc
    return nc


def kernel(x, input_tensor, accuracy_tensor):
    x = np.asarray(x, dtype=np.float32)
    refs = np.ascontiguousarray(np.asarray(input_tensor, dtype=np.float32))
    acc = np.ascontiguousarray(np.asarray(accuracy_tensor, dtype=np.float32))

    nc = _build()
    in_maps = [
        {
            "xq": np.ascontiguousarray(x[i * B_CORE : (i + 1) * B_CORE]),
            "refs": refs,
            "acc": acc,
        }
        for i in range(N_CORES)
    ]
    res = run_bass_kernel_spmd(nc, in_maps, core_ids=list(range(N_CORES)))
    return np.concatenate([res.results[i]["out"] for i in range(N_CORES)])
